# revision 23
# baseline (speedup 1.0000x reference)
"""AttnDecoderRNN single-step on 8 Trainium2 NeuronCores (Bass/Tile).

v3 — tensor-parallel over vocab + sharded GRU/combine:
  - out_W/out_b sharded over vocab (50257 -> 8*6656) in bf16 (fp8-e4m3
    x64 optional via OUTW_DT=fp8); per-core logits via TensorE matvecs,
    exp + partial sum-exp, one AllReduce of the scalar partial sums,
    log-softmax normalization on device.
  - attention fully replicated in bf16 (attn_weights is an output;
    needs the accuracy), softmax without max-subtraction (logits O(1)).
  - combine sharded over H-out (128 rows/core), GRU sharded over the
    contraction dim; gate partials (+bias, on core 0 only) exchanged
    with one AllReduce([1,4096]); every core forms the full h_new.
  - a dummy AllReduce at t=0 absorbs the ~60us ncfw first-collective
    startup; ACT tables (exp/sigmoid/tanh/ln) pre-warmed the same way.

Layouts: vectors are [128, N/128] "pf" (C-order reshape); a matvec
y = x @ W.T runs as sum_f lhsT(x_pf[:, f]) @ slab_f with host-shuffled
slab_f[p, :] = W.T[p*F+f, :]; biases fold in as an extra slab paired
with an e0 one-hot column.
"""
import sys
import os

if "/opt/trn_rl_repo" not in sys.path:
    sys.path.insert(0, "/opt/trn_rl_repo")

import numpy as np
import ml_dtypes

import concourse.bacc as bacc
import concourse.mybir as mybir
import concourse.tile as tile
from concourse import bass_utils

BF16 = ml_dtypes.bfloat16
FP8 = mybir.dt.np(mybir.dt.float8e4)
OUTW_DT = os.environ.get("OUTW_DT", "bf16")  # bf16 | fp8 | fp8dr (DoubleRow)

H = 1024
V = 50257
L = 512
NC = 8
HC = H // NC          # 128 combine rows / GRU contraction elems per core
VPAD = 53248
VC = VPAD // NC       # 6656
NT = 16
TW = VC // NT         # 416 = 8 partitions * 52
FP = VC // 128        # 52
FH = H // 128         # 8
F2H = 2 * H // 128    # 16
FL = L // 128         # 4
NEG = -1.0e30
OWS = 64.0 if OUTW_DT in ("fp8", "fp8dr") else 1.0   # fp8 scale for out_W

_CACHE = {}
LAST_EXEC_NS = None


# ----------------------------------------------------------------- host prep

def _pf(vec, f):
    return np.ascontiguousarray(np.asarray(vec, np.float32).reshape(128, f))


def _slabs(wt, m):
    k = wt.shape[0]
    fk = k // 128
    return np.ascontiguousarray(wt.reshape(128, fk, m).transpose(1, 0, 2))


def _bias_slab(b, m):
    s = np.zeros((1, 128, m), np.float32)
    s[0, 0, :] = b
    return s


def _pack(slab_list, dt=BF16):
    s = np.concatenate(slab_list, axis=0)
    return np.ascontiguousarray(s.transpose(1, 0, 2).reshape(128, -1)).astype(dt)


def prep_inputs(input_tok, hidden, encoder_outputs, emb_table, attn_W, attn_b,
                comb_W, comb_b, gru_Wih, gru_Whh, gru_bih, gru_bhh, out_W, out_b):
    tok = int(np.asarray(input_tok).ravel()[0])
    emb_row = np.asarray(emb_table, np.float32)[tok]
    h0 = np.asarray(hidden, np.float32).reshape(H)
    cat1 = np.concatenate([emb_row, h0])

    attn_W = np.asarray(attn_W, np.float32)
    attn_b = np.asarray(attn_b, np.float32)
    enc = np.asarray(encoder_outputs, np.float32)
    comb_W = np.asarray(comb_W, np.float32)
    comb_b = np.asarray(comb_b, np.float32)
    wih = np.asarray(gru_Wih, np.float32)
    whh = np.asarray(gru_Whh, np.float32)
    bih = np.asarray(gru_bih, np.float32)
    bhh = np.asarray(gru_bhh, np.float32)
    out_W = np.asarray(out_W, np.float32)
    out_b = np.asarray(out_b, np.float32)

    rep = {}
    rep["cat1_bf"] = _pf(cat1, F2H).astype(BF16)
    rep["emb_bf"] = _pf(emb_row, FH).astype(BF16)
    rep["h0_pf"] = _pf(h0, FH)
    e0 = np.zeros((128, 1), np.float32)
    e0[0, 0] = 1.0
    rep["e0_bf"] = e0.astype(BF16)
    rep["attn_w"] = _pack([_slabs(attn_W.T, L), _bias_slab(attn_b, L)])
    rep["enc_w"] = _pack([_slabs(enc, H)])

    owt = np.zeros((H, VPAD), np.float32)
    owt[:, :V] = out_W.T
    ob = np.full(VPAD, NEG, np.float32)
    ob[:V] = out_b

    in_maps = []
    for c in range(NC):
        m = dict(rep)
        hsl = slice(c * HC, (c + 1) * HC)
        m["comb_w"] = _pack([_slabs(comb_W[hsl, :H].T, HC),
                             _slabs(comb_W[hsl, H:].T, HC),
                             _bias_slab(comb_b[hsl], HC)])
        m["wih_w"] = np.ascontiguousarray(wih[:, hsl].T).astype(BF16)
        m["whh_w"] = np.ascontiguousarray(whh[:, hsl].T).astype(BF16)
        m["h0c_bf"] = np.ascontiguousarray(h0[hsl].reshape(128, 1)).astype(BF16)
        # GRU biases only on core 0 (summed by the AllReduce):
        # payload layout [rz (bih+bhh) | n_i (bih) | n_h (bhh)]
        gb = np.zeros((128, 4096), np.float32)
        if c == 0:
            gb[0, 0:2048] = (bih + bhh)[0:2048]
            gb[0, 2048:3072] = bih[2048:3072]
            gb[0, 3072:4096] = bhh[2048:3072]
        m["gbias"] = gb.astype(BF16)

        wt_c = owt[:, c * VC:(c + 1) * VC] * OWS
        if OUTW_DT == "fp8dr":
            # mm tiles of 208 cols: [t32, p, pair, j, n], k = p*8 + 2*pair + j
            # packed per DMA slab of two mm tiles -> [16, 128, 3328]
            m["outw"] = np.ascontiguousarray(
                wt_c.reshape(128, 4, 2, 32, 208).transpose(3, 0, 1, 2, 4)
                .reshape(16, 2, 128, 4 * 2 * 208).transpose(0, 2, 1, 3)
                .reshape(NT, 128, FH * TW)).astype(FP8)
        else:
            m["outw"] = np.ascontiguousarray(
                wt_c.reshape(128, FH, NT, TW).transpose(2, 0, 1, 3).reshape(NT, 128, FH * TW)
            ).astype(FP8 if OUTW_DT == "fp8" else BF16)
        m["outb"] = np.ascontiguousarray(ob[c * VC:(c + 1) * VC].reshape(128, FP))
        in_maps.append(m)
    return in_maps


# ------------------------------------------------------------- device kernel

def build_nc():
    bf = mybir.dt.bfloat16
    f8 = mybir.dt.float8e4
    f32 = mybir.dt.float32
    ACT = mybir.ActivationFunctionType
    OP = mybir.AluOpType

    nc = bacc.Bacc("TRN2", target_bir_lowering=False, debug=False, num_devices=NC)

    i_cat1 = nc.dram_tensor("cat1_bf", [128, F2H], bf, kind="ExternalInput")
    i_emb = nc.dram_tensor("emb_bf", [128, FH], bf, kind="ExternalInput")
    i_h0f = nc.dram_tensor("h0_pf", [128, FH], f32, kind="ExternalInput")
    i_h0c = nc.dram_tensor("h0c_bf", [128, 1], bf, kind="ExternalInput")
    i_e0 = nc.dram_tensor("e0_bf", [128, 1], bf, kind="ExternalInput")
    i_attn = nc.dram_tensor("attn_w", [128, 17 * L], bf, kind="ExternalInput")
    i_enc = nc.dram_tensor("enc_w", [128, FL * H], bf, kind="ExternalInput")
    i_comb = nc.dram_tensor("comb_w", [128, 17 * HC], bf, kind="ExternalInput")
    i_wih = nc.dram_tensor("wih_w", [128, 3 * H], bf, kind="ExternalInput")
    i_whh = nc.dram_tensor("whh_w", [128, 3 * H], bf, kind="ExternalInput")
    i_gb = nc.dram_tensor("gbias", [128, 4096], bf, kind="ExternalInput")
    wdt = f8 if OUTW_DT in ("fp8", "fp8dr") else bf
    i_outw = nc.dram_tensor("outw", [NT, 128, FH * TW], wdt, kind="ExternalInput")
    i_outb = nc.dram_tensor("outb", [128, FP], f32, kind="ExternalInput")

    o_logp = nc.dram_tensor("logp", [128, FP], f32, kind="ExternalOutput")
    o_hnew = nc.dram_tensor("hnew", [128, FH], f32, kind="ExternalOutput")
    o_attnw = nc.dram_tensor("attnw", [1, L], f32, kind="ExternalOutput")
    o_dbg = nc.dram_tensor("dbg", [1, 16], f32, kind="ExternalOutput")

    with tile.TileContext(nc) as tc:
        with tc.tile_pool(name="sb", bufs=1) as sb, \
             tc.tile_pool(name="ps", bufs=1, space="PSUM") as ps, \
             tc.tile_pool(name="dram", bufs=1, space="DRAM") as dram:

            def prow(shape, tag, name):
                pad = [1, 1024] if shape[0] == 1 else [128, 256]
                return ps.tile(shape, f32, tag=tag, padded_shape=pad, name=name)

            # ---- dummy AllReduce right away (absorbs ncfw startup)
            warm_in = dram.tile([1, 4], f32)
            warm_out = dram.tile([NC, 4], f32)
            nc.gpsimd.collective_compute(
                "AllGather", OP.bypass, replica_groups=[list(range(NC))],
                ins=[warm_in.opt()], outs=[warm_out.opt()])

            # ---- ACT table pre-warm (Exp for attention; others staged later)
            warm1 = sb.tile([1, 1], f32)
            nc.vector.memset(warm1[:], 1.0)
            wtmp = sb.tile([1, 1], f32)
            nc.scalar.activation(wtmp[:], warm1[:], ACT.Exp)

            # ---- inputs -> SBUF (all resident), critical-path order
            cat1_bf = sb.tile([128, F2H], bf)
            nc.sync.dma_start(cat1_bf[:], i_cat1[:])
            emb_bf = sb.tile([128, FH], bf)
            nc.sync.dma_start(emb_bf[:], i_emb[:])
            h0_pf = sb.tile([128, FH], f32)
            nc.sync.dma_start(h0_pf[:], i_h0f[:])
            h0c_bf = sb.tile([128, 1], bf)
            nc.sync.dma_start(h0c_bf[:], i_h0c[:])
            e0_bf = sb.tile([128, 1], bf)
            nc.sync.dma_start(e0_bf[:], i_e0[:])
            attn_sb = sb.tile([128, 17 * L], bf)
            for q in range(8):
                s = slice(q * 1088, (q + 1) * 1088)
                nc.sync.dma_start(attn_sb[:, s], i_attn[:, s])
            enc_sb = sb.tile([128, FL * H], bf)
            for q in range(4):
                s = slice(q * 1024, (q + 1) * 1024)
                nc.sync.dma_start(enc_sb[:, s], i_enc[:, s])
            comb_sb = sb.tile([128, 17 * HC], bf)
            nc.sync.dma_start(comb_sb[:], i_comb[:])
            wih_sb = sb.tile([128, 3 * H], bf)
            for q in range(2):
                s = slice(q * 1536, (q + 1) * 1536)
                nc.sync.dma_start(wih_sb[:, s], i_wih[:, s])
            whh_sb = sb.tile([128, 3 * H], bf)
            for q in range(2):
                s = slice(q * 1536, (q + 1) * 1536)
                nc.sync.dma_start(whh_sb[:, s], i_whh[:, s])
            gb_sb = sb.tile([128, 4096], bf)
            nc.sync.dma_start(gb_sb[:], i_gb[:])
            outb_pf = sb.tile([128, FP], f32)
            nc.sync.dma_start(outb_pf[:], i_outb[:])

            outw_tiles = []
            for t in range(NT):
                w = sb.tile([128, FH * TW], wdt, tag="ow", bufs=NT, name=f"ow{t}")
                nc.sync.dma_start(w[:], i_outw[t])
                outw_tiles.append(w)

            ones128 = sb.tile([128, 1], f32)
            nc.vector.memset(ones128[:], 1.0)
            ones_row = sb.tile([1, 128], f32)
            nc.vector.memset(ones_row[:], 1.0)

            # ================= attention (replicated) =================
            att_ps = prow([1, L], "g0", "att_ps")
            for f in range(F2H):
                nc.tensor.matmul(att_ps[:], cat1_bf[:, f:f + 1],
                                 attn_sb[:, f * L:(f + 1) * L],
                                 start=(f == 0), stop=False)
            nc.tensor.matmul(att_ps[:], e0_bf[:], attn_sb[:, 16 * L:17 * L],
                             start=False, stop=True)
            ew_row = sb.tile([1, L], f32)
            sA = sb.tile([1, 1], f32)
            nc.scalar.activation(ew_row[:], att_ps[:], ACT.Exp, accum_out=sA[:])
            rA = sb.tile([1, 1], f32)
            nc.vector.reciprocal(rA[:], sA[:])
            aw_row = sb.tile([1, L], f32)
            nc.vector.tensor_scalar_mul(aw_row[:], ew_row[:], rA[:])
            nc.gpsimd.dma_start(o_attnw[:], aw_row[:])

            nc.scalar.activation(wtmp[:], warm1[:], ACT.Sigmoid)
            nc.scalar.activation(wtmp[:], warm1[:], ACT.Tanh)
            ew_pf = sb.tile([128, FL], f32)
            nc.gpsimd.dma_start(ew_pf[:], ew_row[:])
            ew_bf = sb.tile([128, FL], bf)
            nc.vector.tensor_copy(ew_bf[:], ew_pf[:])

            ctx_ps = prow([1, H], "g1", "ctx_ps")
            for nt2 in range(2):
                cs = slice(nt2 * 512, (nt2 + 1) * 512)
                for f in range(FL):
                    nc.tensor.matmul(ctx_ps[0:1, cs], ew_bf[:, f:f + 1],
                                     enc_sb[:, f * H + nt2 * 512:f * H + (nt2 + 1) * 512],
                                     start=(f == 0), stop=(f == FL - 1))
            ctx_row = sb.tile([1, H], f32)
            nc.scalar.mul(ctx_row[:], ctx_ps[:], rA[0:1, 0:1])
            ctx_pf = sb.tile([128, FH], f32)
            nc.gpsimd.dma_start(ctx_pf[:], ctx_row[:])
            ctx_bf = sb.tile([128, FH], bf)
            nc.vector.tensor_copy(ctx_bf[:], ctx_pf[:])

            # ================= combine (H-out shard) =================
            x_ps = prow([1, HC], "g2", "x_ps")
            for f in range(FH):
                nc.tensor.matmul(x_ps[:], emb_bf[:, f:f + 1],
                                 comb_sb[:, f * HC:(f + 1) * HC],
                                 start=(f == 0), stop=False)
            for f in range(FH):
                nc.tensor.matmul(x_ps[:], ctx_bf[:, f:f + 1],
                                 comb_sb[:, (8 + f) * HC:(9 + f) * HC],
                                 start=False, stop=False)
            nc.tensor.matmul(x_ps[:], e0_bf[:], comb_sb[:, 16 * HC:17 * HC],
                             start=False, stop=True)
            x_row = sb.tile([1, HC], f32)
            nc.scalar.activation(x_row[:], x_ps[:], ACT.Relu)
            x128 = sb.tile([128, 1], f32)
            nc.gpsimd.dma_start(x128[:], x_row[:])
            x128_bf = sb.tile([128, 1], bf)
            nc.vector.tensor_copy(x128_bf[:], x128[:])

            # ================= GRU partials (+bias on core0) =================
            # payload [r (1024) | z (1024) | n_i (1024) | n_h (1024)]
            def part_psum(tag, name, wcol, use_x, use_h, bcol):
                gp = prow([1, H], tag, name)
                for nt2 in range(2):
                    cs = slice(nt2 * 512, (nt2 + 1) * 512)
                    ws = slice(wcol + nt2 * 512, wcol + (nt2 + 1) * 512)
                    first = True
                    if use_x:
                        nc.tensor.matmul(gp[0:1, cs], x128_bf[:], wih_sb[:, ws],
                                         start=True, stop=False)
                        first = False
                    if use_h:
                        nc.tensor.matmul(gp[0:1, cs], h0c_bf[:], whh_sb[:, ws],
                                         start=first, stop=False)
                    nc.tensor.matmul(gp[0:1, cs], e0_bf[:],
                                     gb_sb[:, bcol + nt2 * 512:bcol + (nt2 + 1) * 512],
                                     start=False, stop=True)
                return gp

            r_ps = part_psum("g2", "r_ps", 0, True, True, 0)
            z_ps = part_psum("g3", "z_ps", H, True, True, H)
            ni_ps = part_psum("g0", "ni_ps", 2 * H, True, False, 2 * H)
            nh_ps = part_psum("g1", "nh_ps", 2 * H, False, True, 3 * H)

            pay2 = sb.tile([1, 4096], f32)
            nc.vector.tensor_copy(pay2[0:1, 0:1024], r_ps[:])
            nc.scalar.copy(pay2[0:1, 1024:2048], z_ps[:])
            nc.vector.tensor_copy(pay2[0:1, 2048:3072], ni_ps[:])
            nc.scalar.copy(pay2[0:1, 3072:4096], nh_ps[:])

            cc2_in = dram.tile([1, 4096], f32)
            cc2_out = dram.tile([1, 4096], f32)
            nc.gpsimd.dma_start(cc2_in[:], pay2[:])
            nc.gpsimd.collective_compute(
                "AllReduce", OP.add, replica_groups=[list(range(NC))],
                ins=[cc2_in.opt()], outs=[cc2_out.opt()])

            # gates in pf layout straight from the AllReduce result
            r_pf = sb.tile([128, FH], f32)
            nc.gpsimd.dma_start(r_pf[:], cc2_out[0:1, 0:1024])
            z_pf = sb.tile([128, FH], f32)
            nc.gpsimd.dma_start(z_pf[:], cc2_out[0:1, 1024:2048])
            ni_pf = sb.tile([128, FH], f32)
            nc.gpsimd.dma_start(ni_pf[:], cc2_out[0:1, 2048:3072])
            nh_pf = sb.tile([128, FH], f32)
            nc.gpsimd.dma_start(nh_pf[:], cc2_out[0:1, 3072:4096])

            r_s = sb.tile([128, FH], f32)
            nc.scalar.activation(r_s[:], r_pf[:], ACT.Sigmoid)
            z_s = sb.tile([128, FH], f32)
            nc.scalar.activation(z_s[:], z_pf[:], ACT.Sigmoid)
            rnh = sb.tile([128, FH], f32)
            nc.vector.tensor_mul(rnh[:], r_s[:], nh_pf[:])
            pre_n = sb.tile([128, FH], f32)
            nc.vector.tensor_add(pre_n[:], rnh[:], ni_pf[:])
            n_pf = sb.tile([128, FH], f32)
            nc.scalar.activation(n_pf[:], pre_n[:], ACT.Tanh)
            d_pf = sb.tile([128, FH], f32)
            nc.vector.tensor_sub(d_pf[:], h0_pf[:], n_pf[:])
            zd_pf = sb.tile([128, FH], f32)
            nc.vector.tensor_mul(zd_pf[:], z_s[:], d_pf[:])
            hnew_pf = sb.tile([128, FH], f32)
            nc.vector.tensor_add(hnew_pf[:], n_pf[:], zd_pf[:])
            nc.gpsimd.dma_start(o_hnew[:], hnew_pf[:])
            h_bf = sb.tile([128, FH], wdt)
            nc.vector.tensor_copy(h_bf[:], hnew_pf[:])
            if OUTW_DT == "fp8dr":
                h_dr = sb.tile([128, 128], wdt)
                # col pr*32 + 16*j = h[p*8 + 2*pr + j]
                nc.vector.tensor_copy(h_dr[:, 0:128:32], hnew_pf[:, 0:8:2])
                nc.vector.tensor_copy(h_dr[:, 16:128:32], hnew_pf[:, 1:8:2])

            # ================= output projection (fp8 W, x64) =================
            nc.scalar.activation(wtmp[:], warm1[:], ACT.Exp)
            lg_sb = sb.tile([128, FP], f32)
            for t in range(NT):
                lg_ps = prow([1, TW], f"g{t % 4}", f"lg{t}")
                if OUTW_DT == "fp8dr":
                    wv = outw_tiles[t].rearrange("p (u pr j n) -> p u pr j n",
                                                 u=2, pr=4, j=2)
                    for u in range(2):
                        sub = lg_ps[0:1, u * 208:(u + 1) * 208]
                        for pr in range(4):
                            lhsT = h_dr[:, pr * 32:pr * 32 + 17:16]
                            nc.tensor.matmul(sub, lhsT,
                                             wv[:, u, pr, :, :],
                                             start=(pr == 0), stop=(pr == 3),
                                             perf_mode=mybir.MatmulPerfMode.DoubleRow)
                else:
                    for f in range(FH):
                        nc.tensor.matmul(lg_ps[:], h_bf[:, f:f + 1],
                                         outw_tiles[t][:, f * TW:(f + 1) * TW],
                                         start=(f == 0), stop=(f == FH - 1))
                lg_row = sb.tile([1, TW], f32, tag="lgrow", bufs=4, name=f"lgr{t}")
                if t % 2 == 0:
                    nc.vector.tensor_copy(lg_row[:], lg_ps[:])
                else:
                    nc.scalar.copy(lg_row[:], lg_ps[:])
                nc.gpsimd.dma_start(lg_sb[8 * t:8 * (t + 1), :], lg_row[:])

            # lb = lg / OWS + out_b ; exp + row sums
            lb_sb = sb.tile([128, FP], f32)
            nc.vector.scalar_tensor_tensor(lb_sb[:], lg_sb[:], 1.0 / OWS, outb_pf[:],
                                           op0=mybir.AluOpType.mult,
                                           op1=mybir.AluOpType.add)
            ex_sb = sb.tile([128, FP], f32)
            rowsum = sb.tile([128, 1], f32)
            nc.scalar.activation(ex_sb[:], lb_sb[:], ACT.Exp, accum_out=rowsum[:])

            sum_ps = prow([1, 1], "g1", "sum_ps")
            nc.tensor.matmul(sum_ps[:], ones128[:], rowsum[:], start=True, stop=True)
            s_sb = sb.tile([1, 1], f32)
            nc.scalar.copy(s_sb[:], sum_ps[:])

            nc.scalar.activation(wtmp[:], warm1[:], ACT.Ln)
            cc3_in = dram.tile([1, 1], f32)
            cc3_out = dram.tile([1, 1], f32)
            nc.gpsimd.dma_start(cc3_in[:], s_sb[:])
            nc.gpsimd.collective_compute(
                "AllReduce", OP.add, replica_groups=[list(range(NC))],
                ins=[cc3_in.opt()], outs=[cc3_out.opt()])
            S_sb = sb.tile([1, 1], f32)
            nc.gpsimd.dma_start(S_sb[:], cc3_out[:])

            delta = sb.tile([1, 1], f32)
            nc.scalar.activation(delta[:], S_sb[:], ACT.Ln)
            bc_ps = prow([128, 1], "g2", "bc_ps")
            nc.tensor.matmul(bc_ps[:], ones_row[:], delta[:], start=True, stop=True)
            bc_sb = sb.tile([128, 1], f32)
            nc.vector.tensor_copy(bc_sb[:], bc_ps[:])

            logp_sb = sb.tile([128, FP], f32)
            nc.vector.tensor_scalar(logp_sb[:], lb_sb[:], bc_sb[:], None,
                                    op0=mybir.AluOpType.subtract)
            nc.gpsimd.dma_start(o_logp[:], logp_sb[:])

            warm_sb = sb.tile([NC, 4], f32)
            nc.sync.dma_start(warm_sb[:], warm_out[:])
            nc.sync.dma_start(o_dbg[0:1, 0:4], warm_sb[0:1, :])

    nc.compile()
    return nc


# ------------------------------------------------------------------- runner

def _get_nc():
    if "nc" not in _CACHE:
        _CACHE["nc"] = build_nc()
    return _CACHE["nc"]


def kernel(**inputs):
    global LAST_EXEC_NS
    in_maps = prep_inputs(**inputs)
    nc = _get_nc()
    trace = bool(int(os.environ.get("KERNEL_TRACE", "0")))
    if trace:
        try:
            from bass_exec import run_spmd_traced
            res = run_spmd_traced(nc, in_maps, NC)
        except Exception:
            res = bass_utils.run_bass_kernel_spmd(
                nc, in_maps, core_ids=list(range(NC)))
    else:
        res = bass_utils.run_bass_kernel_spmd(
            nc, in_maps, core_ids=list(range(NC)))
    LAST_EXEC_NS = res.exec_time_ns

    logp = np.concatenate(
        [res.results[c]["logp"].reshape(VC) for c in range(NC)])[:V][None, :]
    hnew = res.results[0]["hnew"].reshape(1, 1, H)
    attnw = res.results[0]["attnw"].reshape(1, L)
    return (np.ascontiguousarray(logp.astype(np.float32)),
            np.ascontiguousarray(hnew.astype(np.float32)),
            np.ascontiguousarray(attnw.astype(np.float32)))


# revision 24
# speedup vs baseline: 1.5534x; 1.5534x over previous
"""AttnDecoderRNN single-step on 8 Trainium2 NeuronCores (Bass/Tile).

v3 — tensor-parallel over vocab + sharded GRU/combine:
  - out_W/out_b sharded over vocab (50257 -> 8*6656) in fp8-e4m3 (x64
    scale); per-core logits via TensorE DoubleRow matvecs (K=256/pass,
    h interleaved as 16-strided fp8 plane pairs), exp + partial
    sum-exp, one AllReduce of the scalar partial sums, log-softmax
    normalization on device. OUTW_DT=bf16|fp8 fall back available.
  - attention fully replicated in bf16 (attn_weights is an output;
    needs the accuracy), softmax without max-subtraction (logits O(1)).
  - combine sharded over H-out (128 rows/core), GRU sharded over the
    contraction dim; gate partials (+bias, on core 0 only) exchanged
    with one AllReduce([1,4096]); every core forms the full h_new.
  - a dummy AllReduce at t=0 absorbs the ~60us ncfw first-collective
    startup; ACT tables (exp/sigmoid/tanh/ln) pre-warmed the same way.

Layouts: vectors are [128, N/128] "pf" (C-order reshape); a matvec
y = x @ W.T runs as sum_f lhsT(x_pf[:, f]) @ slab_f with host-shuffled
slab_f[p, :] = W.T[p*F+f, :]; biases fold in as an extra slab paired
with an e0 one-hot column.
"""
import sys
import os

if "/opt/trn_rl_repo" not in sys.path:
    sys.path.insert(0, "/opt/trn_rl_repo")

import numpy as np
import ml_dtypes

import concourse.bacc as bacc
import concourse.mybir as mybir
import concourse.tile as tile
from concourse import bass_utils

BF16 = ml_dtypes.bfloat16
FP8 = mybir.dt.np(mybir.dt.float8e4)
OUTW_DT = os.environ.get("OUTW_DT", "fp8dr")  # fp8dr (DoubleRow, default) | fp8 | bf16

H = 1024
V = 50257
L = 512
NC = 8
HC = H // NC          # 128 combine rows / GRU contraction elems per core
VPAD = 53248
VC = VPAD // NC       # 6656
NT = 16
TW = VC // NT         # 416 = 8 partitions * 52
FP = VC // 128        # 52
FH = H // 128         # 8
F2H = 2 * H // 128    # 16
FL = L // 128         # 4
NEG = -1.0e30
OWS = 64.0 if OUTW_DT in ("fp8", "fp8dr") else 1.0   # fp8 scale for out_W

_CACHE = {}
LAST_EXEC_NS = None


# ----------------------------------------------------------------- host prep

def _pf(vec, f):
    return np.ascontiguousarray(np.asarray(vec, np.float32).reshape(128, f))


def _slabs(wt, m):
    k = wt.shape[0]
    fk = k // 128
    return np.ascontiguousarray(wt.reshape(128, fk, m).transpose(1, 0, 2))


def _bias_slab(b, m):
    s = np.zeros((1, 128, m), np.float32)
    s[0, 0, :] = b
    return s


def _pack(slab_list, dt=BF16):
    s = np.concatenate(slab_list, axis=0)
    return np.ascontiguousarray(s.transpose(1, 0, 2).reshape(128, -1)).astype(dt)


def prep_inputs(input_tok, hidden, encoder_outputs, emb_table, attn_W, attn_b,
                comb_W, comb_b, gru_Wih, gru_Whh, gru_bih, gru_bhh, out_W, out_b):
    tok = int(np.asarray(input_tok).ravel()[0])
    emb_row = np.asarray(emb_table, np.float32)[tok]
    h0 = np.asarray(hidden, np.float32).reshape(H)
    cat1 = np.concatenate([emb_row, h0])

    attn_W = np.asarray(attn_W, np.float32)
    attn_b = np.asarray(attn_b, np.float32)
    enc = np.asarray(encoder_outputs, np.float32)
    comb_W = np.asarray(comb_W, np.float32)
    comb_b = np.asarray(comb_b, np.float32)
    wih = np.asarray(gru_Wih, np.float32)
    whh = np.asarray(gru_Whh, np.float32)
    bih = np.asarray(gru_bih, np.float32)
    bhh = np.asarray(gru_bhh, np.float32)
    out_W = np.asarray(out_W, np.float32)
    out_b = np.asarray(out_b, np.float32)

    rep = {}
    rep["cat1_bf"] = _pf(cat1, F2H).astype(BF16)
    rep["emb_bf"] = _pf(emb_row, FH).astype(BF16)
    rep["h0_pf"] = _pf(h0, FH)
    e0 = np.zeros((128, 1), np.float32)
    e0[0, 0] = 1.0
    rep["e0_bf"] = e0.astype(BF16)
    rep["attn_w"] = _pack([_slabs(attn_W.T, L), _bias_slab(attn_b, L)])
    rep["enc_w"] = _pack([_slabs(enc, H)])

    owt = np.zeros((H, VPAD), np.float32)
    owt[:, :V] = out_W.T
    ob = np.full(VPAD, NEG, np.float32)
    ob[:V] = out_b

    in_maps = []
    for c in range(NC):
        m = dict(rep)
        hsl = slice(c * HC, (c + 1) * HC)
        m["comb_w"] = _pack([_slabs(comb_W[hsl, :H].T, HC),
                             _slabs(comb_W[hsl, H:].T, HC),
                             _bias_slab(comb_b[hsl], HC)])
        m["wih_w"] = np.ascontiguousarray(wih[:, hsl].T).astype(BF16)
        m["whh_w"] = np.ascontiguousarray(whh[:, hsl].T).astype(BF16)
        m["h0c_bf"] = np.ascontiguousarray(h0[hsl].reshape(128, 1)).astype(BF16)
        # GRU biases only on core 0 (summed by the AllReduce):
        # payload layout [rz (bih+bhh) | n_i (bih) | n_h (bhh)]
        gb = np.zeros((128, 4096), np.float32)
        if c == 0:
            gb[0, 0:2048] = (bih + bhh)[0:2048]
            gb[0, 2048:3072] = bih[2048:3072]
            gb[0, 3072:4096] = bhh[2048:3072]
        m["gbias"] = gb.astype(BF16)

        wt_c = owt[:, c * VC:(c + 1) * VC] * OWS
        if OUTW_DT == "fp8dr":
            # mm tiles of 208 cols: [t32, p, pair, j, n], k = p*8 + 2*pair + j
            # packed per DMA slab of two mm tiles -> [16, 128, 3328]
            m["outw"] = np.ascontiguousarray(
                wt_c.reshape(128, 4, 2, 32, 208).transpose(3, 0, 1, 2, 4)
                .reshape(16, 2, 128, 4 * 2 * 208).transpose(0, 2, 1, 3)
                .reshape(NT, 128, FH * TW)).astype(FP8)
        else:
            m["outw"] = np.ascontiguousarray(
                wt_c.reshape(128, FH, NT, TW).transpose(2, 0, 1, 3).reshape(NT, 128, FH * TW)
            ).astype(FP8 if OUTW_DT == "fp8" else BF16)
        m["outb"] = np.ascontiguousarray(ob[c * VC:(c + 1) * VC].reshape(128, FP))
        in_maps.append(m)
    return in_maps


# ------------------------------------------------------------- device kernel

def build_nc():
    bf = mybir.dt.bfloat16
    f8 = mybir.dt.float8e4
    f32 = mybir.dt.float32
    ACT = mybir.ActivationFunctionType
    OP = mybir.AluOpType

    nc = bacc.Bacc("TRN2", target_bir_lowering=False, debug=False, num_devices=NC)

    i_cat1 = nc.dram_tensor("cat1_bf", [128, F2H], bf, kind="ExternalInput")
    i_emb = nc.dram_tensor("emb_bf", [128, FH], bf, kind="ExternalInput")
    i_h0f = nc.dram_tensor("h0_pf", [128, FH], f32, kind="ExternalInput")
    i_h0c = nc.dram_tensor("h0c_bf", [128, 1], bf, kind="ExternalInput")
    i_e0 = nc.dram_tensor("e0_bf", [128, 1], bf, kind="ExternalInput")
    i_attn = nc.dram_tensor("attn_w", [128, 17 * L], bf, kind="ExternalInput")
    i_enc = nc.dram_tensor("enc_w", [128, FL * H], bf, kind="ExternalInput")
    i_comb = nc.dram_tensor("comb_w", [128, 17 * HC], bf, kind="ExternalInput")
    i_wih = nc.dram_tensor("wih_w", [128, 3 * H], bf, kind="ExternalInput")
    i_whh = nc.dram_tensor("whh_w", [128, 3 * H], bf, kind="ExternalInput")
    i_gb = nc.dram_tensor("gbias", [128, 4096], bf, kind="ExternalInput")
    wdt = f8 if OUTW_DT in ("fp8", "fp8dr") else bf
    i_outw = nc.dram_tensor("outw", [NT, 128, FH * TW], wdt, kind="ExternalInput")
    i_outb = nc.dram_tensor("outb", [128, FP], f32, kind="ExternalInput")

    o_logp = nc.dram_tensor("logp", [128, FP], f32, kind="ExternalOutput")
    o_hnew = nc.dram_tensor("hnew", [128, FH], f32, kind="ExternalOutput")
    o_attnw = nc.dram_tensor("attnw", [1, L], f32, kind="ExternalOutput")
    o_dbg = nc.dram_tensor("dbg", [1, 16], f32, kind="ExternalOutput")

    with tile.TileContext(nc) as tc:
        with tc.tile_pool(name="sb", bufs=1) as sb, \
             tc.tile_pool(name="ps", bufs=1, space="PSUM") as ps, \
             tc.tile_pool(name="dram", bufs=1, space="DRAM") as dram:

            def prow(shape, tag, name):
                pad = [1, 1024] if shape[0] == 1 else [128, 256]
                return ps.tile(shape, f32, tag=tag, padded_shape=pad, name=name)

            # ---- dummy AllReduce right away (absorbs ncfw startup)
            warm_in = dram.tile([1, 4], f32)
            warm_out = dram.tile([NC, 4], f32)
            nc.gpsimd.collective_compute(
                "AllGather", OP.bypass, replica_groups=[list(range(NC))],
                ins=[warm_in.opt()], outs=[warm_out.opt()])

            # ---- ACT table pre-warm (Exp for attention; others staged later)
            warm1 = sb.tile([1, 1], f32)
            nc.vector.memset(warm1[:], 1.0)
            wtmp = sb.tile([1, 1], f32)
            nc.scalar.activation(wtmp[:], warm1[:], ACT.Exp)

            # ---- inputs -> SBUF (all resident), critical-path order
            cat1_bf = sb.tile([128, F2H], bf)
            nc.sync.dma_start(cat1_bf[:], i_cat1[:])
            emb_bf = sb.tile([128, FH], bf)
            nc.sync.dma_start(emb_bf[:], i_emb[:])
            h0_pf = sb.tile([128, FH], f32)
            nc.sync.dma_start(h0_pf[:], i_h0f[:])
            h0c_bf = sb.tile([128, 1], bf)
            nc.sync.dma_start(h0c_bf[:], i_h0c[:])
            e0_bf = sb.tile([128, 1], bf)
            nc.sync.dma_start(e0_bf[:], i_e0[:])
            attn_sb = sb.tile([128, 17 * L], bf)
            for q in range(8):
                s = slice(q * 1088, (q + 1) * 1088)
                nc.sync.dma_start(attn_sb[:, s], i_attn[:, s])
            enc_sb = sb.tile([128, FL * H], bf)
            for q in range(4):
                s = slice(q * 1024, (q + 1) * 1024)
                nc.sync.dma_start(enc_sb[:, s], i_enc[:, s])
            comb_sb = sb.tile([128, 17 * HC], bf)
            nc.sync.dma_start(comb_sb[:], i_comb[:])
            wih_sb = sb.tile([128, 3 * H], bf)
            for q in range(2):
                s = slice(q * 1536, (q + 1) * 1536)
                nc.sync.dma_start(wih_sb[:, s], i_wih[:, s])
            whh_sb = sb.tile([128, 3 * H], bf)
            for q in range(2):
                s = slice(q * 1536, (q + 1) * 1536)
                nc.sync.dma_start(whh_sb[:, s], i_whh[:, s])
            gb_sb = sb.tile([128, 4096], bf)
            nc.sync.dma_start(gb_sb[:], i_gb[:])
            outb_pf = sb.tile([128, FP], f32)
            nc.sync.dma_start(outb_pf[:], i_outb[:])

            outw_tiles = []
            for t in range(NT):
                w = sb.tile([128, FH * TW], wdt, tag="ow", bufs=NT, name=f"ow{t}")
                nc.sync.dma_start(w[:], i_outw[t])
                outw_tiles.append(w)

            ones128 = sb.tile([128, 1], f32)
            nc.vector.memset(ones128[:], 1.0)
            ones_row = sb.tile([1, 128], f32)
            nc.vector.memset(ones_row[:], 1.0)

            # ================= attention (replicated) =================
            att_ps = prow([1, L], "g0", "att_ps")
            for f in range(F2H):
                nc.tensor.matmul(att_ps[:], cat1_bf[:, f:f + 1],
                                 attn_sb[:, f * L:(f + 1) * L],
                                 start=(f == 0), stop=False)
            nc.tensor.matmul(att_ps[:], e0_bf[:], attn_sb[:, 16 * L:17 * L],
                             start=False, stop=True)
            ew_row = sb.tile([1, L], f32)
            sA = sb.tile([1, 1], f32)
            nc.scalar.activation(ew_row[:], att_ps[:], ACT.Exp, accum_out=sA[:])
            rA = sb.tile([1, 1], f32)
            nc.vector.reciprocal(rA[:], sA[:])
            aw_row = sb.tile([1, L], f32)
            nc.vector.tensor_scalar_mul(aw_row[:], ew_row[:], rA[:])
            nc.gpsimd.dma_start(o_attnw[:], aw_row[:])

            nc.scalar.activation(wtmp[:], warm1[:], ACT.Sigmoid)
            nc.scalar.activation(wtmp[:], warm1[:], ACT.Tanh)
            ew_pf = sb.tile([128, FL], f32)
            nc.gpsimd.dma_start(ew_pf[:], ew_row[:])
            ew_bf = sb.tile([128, FL], bf)
            nc.vector.tensor_copy(ew_bf[:], ew_pf[:])

            ctx_ps = prow([1, H], "g1", "ctx_ps")
            for nt2 in range(2):
                cs = slice(nt2 * 512, (nt2 + 1) * 512)
                for f in range(FL):
                    nc.tensor.matmul(ctx_ps[0:1, cs], ew_bf[:, f:f + 1],
                                     enc_sb[:, f * H + nt2 * 512:f * H + (nt2 + 1) * 512],
                                     start=(f == 0), stop=(f == FL - 1))
            ctx_row = sb.tile([1, H], f32)
            nc.scalar.mul(ctx_row[:], ctx_ps[:], rA[0:1, 0:1])
            ctx_pf = sb.tile([128, FH], f32)
            nc.gpsimd.dma_start(ctx_pf[:], ctx_row[:])
            ctx_bf = sb.tile([128, FH], bf)
            nc.vector.tensor_copy(ctx_bf[:], ctx_pf[:])

            # ================= combine (H-out shard) =================
            x_ps = prow([1, HC], "g2", "x_ps")
            for f in range(FH):
                nc.tensor.matmul(x_ps[:], emb_bf[:, f:f + 1],
                                 comb_sb[:, f * HC:(f + 1) * HC],
                                 start=(f == 0), stop=False)
            for f in range(FH):
                nc.tensor.matmul(x_ps[:], ctx_bf[:, f:f + 1],
                                 comb_sb[:, (8 + f) * HC:(9 + f) * HC],
                                 start=False, stop=False)
            nc.tensor.matmul(x_ps[:], e0_bf[:], comb_sb[:, 16 * HC:17 * HC],
                             start=False, stop=True)
            x_row = sb.tile([1, HC], f32)
            nc.scalar.activation(x_row[:], x_ps[:], ACT.Relu)
            x128 = sb.tile([128, 1], f32)
            nc.gpsimd.dma_start(x128[:], x_row[:])
            x128_bf = sb.tile([128, 1], bf)
            nc.vector.tensor_copy(x128_bf[:], x128[:])

            # ================= GRU partials (+bias on core0) =================
            # payload [r (1024) | z (1024) | n_i (1024) | n_h (1024)]
            def part_psum(tag, name, wcol, use_x, use_h, bcol):
                gp = prow([1, H], tag, name)
                for nt2 in range(2):
                    cs = slice(nt2 * 512, (nt2 + 1) * 512)
                    ws = slice(wcol + nt2 * 512, wcol + (nt2 + 1) * 512)
                    first = True
                    if use_x:
                        nc.tensor.matmul(gp[0:1, cs], x128_bf[:], wih_sb[:, ws],
                                         start=True, stop=False)
                        first = False
                    if use_h:
                        nc.tensor.matmul(gp[0:1, cs], h0c_bf[:], whh_sb[:, ws],
                                         start=first, stop=False)
                    nc.tensor.matmul(gp[0:1, cs], e0_bf[:],
                                     gb_sb[:, bcol + nt2 * 512:bcol + (nt2 + 1) * 512],
                                     start=False, stop=True)
                return gp

            r_ps = part_psum("g2", "r_ps", 0, True, True, 0)
            z_ps = part_psum("g3", "z_ps", H, True, True, H)
            ni_ps = part_psum("g0", "ni_ps", 2 * H, True, False, 2 * H)
            nh_ps = part_psum("g1", "nh_ps", 2 * H, False, True, 3 * H)

            pay2 = sb.tile([1, 4096], f32)
            nc.vector.tensor_copy(pay2[0:1, 0:1024], r_ps[:])
            nc.scalar.copy(pay2[0:1, 1024:2048], z_ps[:])
            nc.vector.tensor_copy(pay2[0:1, 2048:3072], ni_ps[:])
            nc.scalar.copy(pay2[0:1, 3072:4096], nh_ps[:])

            cc2_in = dram.tile([1, 4096], f32)
            cc2_out = dram.tile([1, 4096], f32)
            nc.gpsimd.dma_start(cc2_in[:], pay2[:])
            nc.gpsimd.collective_compute(
                "AllReduce", OP.add, replica_groups=[list(range(NC))],
                ins=[cc2_in.opt()], outs=[cc2_out.opt()])

            # gates in pf layout straight from the AllReduce result
            r_pf = sb.tile([128, FH], f32)
            nc.gpsimd.dma_start(r_pf[:], cc2_out[0:1, 0:1024])
            z_pf = sb.tile([128, FH], f32)
            nc.gpsimd.dma_start(z_pf[:], cc2_out[0:1, 1024:2048])
            ni_pf = sb.tile([128, FH], f32)
            nc.gpsimd.dma_start(ni_pf[:], cc2_out[0:1, 2048:3072])
            nh_pf = sb.tile([128, FH], f32)
            nc.gpsimd.dma_start(nh_pf[:], cc2_out[0:1, 3072:4096])

            r_s = sb.tile([128, FH], f32)
            nc.scalar.activation(r_s[:], r_pf[:], ACT.Sigmoid)
            z_s = sb.tile([128, FH], f32)
            nc.scalar.activation(z_s[:], z_pf[:], ACT.Sigmoid)
            rnh = sb.tile([128, FH], f32)
            nc.vector.tensor_mul(rnh[:], r_s[:], nh_pf[:])
            pre_n = sb.tile([128, FH], f32)
            nc.vector.tensor_add(pre_n[:], rnh[:], ni_pf[:])
            n_pf = sb.tile([128, FH], f32)
            nc.scalar.activation(n_pf[:], pre_n[:], ACT.Tanh)
            d_pf = sb.tile([128, FH], f32)
            nc.vector.tensor_sub(d_pf[:], h0_pf[:], n_pf[:])
            zd_pf = sb.tile([128, FH], f32)
            nc.vector.tensor_mul(zd_pf[:], z_s[:], d_pf[:])
            hnew_pf = sb.tile([128, FH], f32)
            nc.vector.tensor_add(hnew_pf[:], n_pf[:], zd_pf[:])
            nc.gpsimd.dma_start(o_hnew[:], hnew_pf[:])
            h_bf = sb.tile([128, FH], wdt)
            nc.vector.tensor_copy(h_bf[:], hnew_pf[:])
            if OUTW_DT == "fp8dr":
                h_dr = sb.tile([128, 128], wdt)
                # col pr*32 + 16*j = h[p*8 + 2*pr + j]
                nc.vector.tensor_copy(h_dr[:, 0:128:32], hnew_pf[:, 0:8:2])
                nc.vector.tensor_copy(h_dr[:, 16:128:32], hnew_pf[:, 1:8:2])

            # ================= output projection (fp8 W, x64) =================
            nc.scalar.activation(wtmp[:], warm1[:], ACT.Exp)
            lg_sb = sb.tile([128, FP], f32)
            for t in range(NT):
                lg_ps = prow([1, TW], f"g{t % 4}", f"lg{t}")
                if OUTW_DT == "fp8dr":
                    wv = outw_tiles[t].rearrange("p (u pr j n) -> p u pr j n",
                                                 u=2, pr=4, j=2)
                    for u in range(2):
                        sub = lg_ps[0:1, u * 208:(u + 1) * 208]
                        for pr in range(4):
                            lhsT = h_dr[:, pr * 32:pr * 32 + 17:16]
                            nc.tensor.matmul(sub, lhsT,
                                             wv[:, u, pr, :, :],
                                             start=(pr == 0), stop=(pr == 3),
                                             perf_mode=mybir.MatmulPerfMode.DoubleRow)
                else:
                    for f in range(FH):
                        nc.tensor.matmul(lg_ps[:], h_bf[:, f:f + 1],
                                         outw_tiles[t][:, f * TW:(f + 1) * TW],
                                         start=(f == 0), stop=(f == FH - 1))
                lg_row = sb.tile([1, TW], f32, tag="lgrow", bufs=4, name=f"lgr{t}")
                if t % 2 == 0:
                    nc.vector.tensor_copy(lg_row[:], lg_ps[:])
                else:
                    nc.scalar.copy(lg_row[:], lg_ps[:])
                nc.gpsimd.dma_start(lg_sb[8 * t:8 * (t + 1), :], lg_row[:])

            # lb = lg / OWS + out_b ; exp + row sums
            lb_sb = sb.tile([128, FP], f32)
            nc.vector.scalar_tensor_tensor(lb_sb[:], lg_sb[:], 1.0 / OWS, outb_pf[:],
                                           op0=mybir.AluOpType.mult,
                                           op1=mybir.AluOpType.add)
            ex_sb = sb.tile([128, FP], f32)
            rowsum = sb.tile([128, 1], f32)
            nc.scalar.activation(ex_sb[:], lb_sb[:], ACT.Exp, accum_out=rowsum[:])

            sum_ps = prow([1, 1], "g1", "sum_ps")
            nc.tensor.matmul(sum_ps[:], ones128[:], rowsum[:], start=True, stop=True)
            s_sb = sb.tile([1, 1], f32)
            nc.scalar.copy(s_sb[:], sum_ps[:])

            nc.scalar.activation(wtmp[:], warm1[:], ACT.Ln)
            cc3_in = dram.tile([1, 1], f32)
            cc3_out = dram.tile([1, 1], f32)
            nc.gpsimd.dma_start(cc3_in[:], s_sb[:])
            nc.gpsimd.collective_compute(
                "AllReduce", OP.add, replica_groups=[list(range(NC))],
                ins=[cc3_in.opt()], outs=[cc3_out.opt()])
            S_sb = sb.tile([1, 1], f32)
            nc.gpsimd.dma_start(S_sb[:], cc3_out[:])

            delta = sb.tile([1, 1], f32)
            nc.scalar.activation(delta[:], S_sb[:], ACT.Ln)
            bc_ps = prow([128, 1], "g2", "bc_ps")
            nc.tensor.matmul(bc_ps[:], ones_row[:], delta[:], start=True, stop=True)
            bc_sb = sb.tile([128, 1], f32)
            nc.vector.tensor_copy(bc_sb[:], bc_ps[:])

            logp_sb = sb.tile([128, FP], f32)
            nc.vector.tensor_scalar(logp_sb[:], lb_sb[:], bc_sb[:], None,
                                    op0=mybir.AluOpType.subtract)
            nc.gpsimd.dma_start(o_logp[:], logp_sb[:])

            warm_sb = sb.tile([NC, 4], f32)
            nc.sync.dma_start(warm_sb[:], warm_out[:])
            nc.sync.dma_start(o_dbg[0:1, 0:4], warm_sb[0:1, :])

    nc.compile()
    return nc


# ------------------------------------------------------------------- runner

def _get_nc():
    if "nc" not in _CACHE:
        _CACHE["nc"] = build_nc()
    return _CACHE["nc"]


def kernel(**inputs):
    global LAST_EXEC_NS
    in_maps = prep_inputs(**inputs)
    nc = _get_nc()
    trace = bool(int(os.environ.get("KERNEL_TRACE", "0")))
    if trace:
        try:
            from bass_exec import run_spmd_traced
            res = run_spmd_traced(nc, in_maps, NC)
        except Exception:
            res = bass_utils.run_bass_kernel_spmd(
                nc, in_maps, core_ids=list(range(NC)))
    else:
        res = bass_utils.run_bass_kernel_spmd(
            nc, in_maps, core_ids=list(range(NC)))
    LAST_EXEC_NS = res.exec_time_ns

    logp = np.concatenate(
        [res.results[c]["logp"].reshape(VC) for c in range(NC)])[:V][None, :]
    hnew = res.results[0]["hnew"].reshape(1, 1, H)
    attnw = res.results[0]["attnw"].reshape(1, L)
    return (np.ascontiguousarray(logp.astype(np.float32)),
            np.ascontiguousarray(hnew.astype(np.float32)),
            np.ascontiguousarray(attnw.astype(np.float32)))


# revision 29
# speedup vs baseline: 1.7596x; 1.1327x over previous
"""AttnDecoderRNN single-step on 8 Trainium2 NeuronCores (Bass/Tile).

v3 — tensor-parallel over vocab + sharded GRU/combine:
  - out_W/out_b sharded over vocab (50257 -> 8*6656) in fp8-e4m3 (x64
    scale); per-core logits via TensorE DoubleRow matvecs (K=256/pass,
    h interleaved as 16-strided fp8 plane pairs), exp + partial
    sum-exp, one AllReduce of the scalar partial sums, log-softmax
    normalization on device. OUTW_DT=bf16|fp8 fall back available.
  - attention fully replicated in bf16 (attn_weights is an output;
    needs the accuracy), softmax without max-subtraction (logits O(1)).
  - combine replicated (hidden in the DMA shadow); GRU sharded by
    OUTPUT rows (128 h_new elements per core): gates computed fully
    per-core BEFORE any collective, then one tiny AllGather([1,128]
    fp8) exchanges the finished h_new slices, landing directly in the
    matvec layouts. h_new output is written as per-core slices.
  - a dummy AllGather at t=0 absorbs the ~60us ncfw first-collective
    startup; ACT tables (exp/sigmoid/tanh/ln) staged into idle windows.

Layouts: vectors are [128, N/128] "pf" (C-order reshape); a matvec
y = x @ W.T runs as sum_f lhsT(x_pf[:, f]) @ slab_f with host-shuffled
slab_f[p, :] = W.T[p*F+f, :]; biases fold in as an extra slab paired
with an e0 one-hot column.
"""
import sys
import os

if "/opt/trn_rl_repo" not in sys.path:
    sys.path.insert(0, "/opt/trn_rl_repo")

import numpy as np
import ml_dtypes

import concourse.bacc as bacc
import concourse.mybir as mybir
import concourse.tile as tile
from concourse import bass_utils

BF16 = ml_dtypes.bfloat16
FP8 = mybir.dt.np(mybir.dt.float8e4)
OUTW_DT = os.environ.get("OUTW_DT", "fp8dr")  # fp8dr (DoubleRow, default) | fp8 | bf16

H = 1024
V = 50257
L = 512
NC = 8
HC = H // NC          # 128 combine rows / GRU contraction elems per core
VPAD = 53248
VC = VPAD // NC       # 6656
NT = 16
TW = VC // NT         # 416 = 8 partitions * 52
FP = VC // 128        # 52
FH = H // 128         # 8
F2H = 2 * H // 128    # 16
FL = L // 128         # 4
NEG = -1.0e30
OWS = 64.0 if OUTW_DT in ("fp8", "fp8dr") else 1.0   # fp8 scale for out_W

_CACHE = {}
LAST_EXEC_NS = None


# ----------------------------------------------------------------- host prep

def _pf(vec, f):
    return np.ascontiguousarray(np.asarray(vec, np.float32).reshape(128, f))


def _slabs(wt, m):
    k = wt.shape[0]
    fk = k // 128
    return np.ascontiguousarray(wt.reshape(128, fk, m).transpose(1, 0, 2))


def _bias_slab(b, m):
    s = np.zeros((1, 128, m), np.float32)
    s[0, 0, :] = b
    return s


def _pack(slab_list, dt=BF16):
    s = np.concatenate(slab_list, axis=0)
    return np.ascontiguousarray(s.transpose(1, 0, 2).reshape(128, -1)).astype(dt)


def prep_inputs(input_tok, hidden, encoder_outputs, emb_table, attn_W, attn_b,
                comb_W, comb_b, gru_Wih, gru_Whh, gru_bih, gru_bhh, out_W, out_b):
    tok = int(np.asarray(input_tok).ravel()[0])
    emb_row = np.asarray(emb_table, np.float32)[tok]
    h0 = np.asarray(hidden, np.float32).reshape(H)
    cat1 = np.concatenate([emb_row, h0])

    attn_W = np.asarray(attn_W, np.float32)
    attn_b = np.asarray(attn_b, np.float32)
    enc = np.asarray(encoder_outputs, np.float32)
    comb_W = np.asarray(comb_W, np.float32)
    comb_b = np.asarray(comb_b, np.float32)
    wih = np.asarray(gru_Wih, np.float32)
    whh = np.asarray(gru_Whh, np.float32)
    bih = np.asarray(gru_bih, np.float32)
    bhh = np.asarray(gru_bhh, np.float32)
    out_W = np.asarray(out_W, np.float32)
    out_b = np.asarray(out_b, np.float32)

    rep = {}
    rep["cat1_bf"] = _pf(cat1, F2H).astype(BF16)
    rep["emb_bf"] = _pf(emb_row, FH).astype(BF16)
    e0 = np.zeros((128, 1), np.float32)
    e0[0, 0] = 1.0
    rep["e0_bf"] = e0.astype(BF16)
    rep["attn_w"] = _pack([_slabs(attn_W.T, L), _bias_slab(attn_b, L)])
    rep["enc_w"] = _pack([_slabs(enc, H)])
    cwt = comb_W.T
    rep["comb_w"] = _pack([_slabs(cwt[:H], H), _slabs(cwt[H:], H),
                           _bias_slab(comb_b, H)])
    rep["h0_bf"] = _pf(h0, FH).astype(BF16)

    owt = np.zeros((H, VPAD), np.float32)
    owt[:, :V] = out_W.T
    ob = np.full(VPAD, NEG, np.float32)
    ob[:V] = out_b

    in_maps = []
    for c in range(NC):
        m = dict(rep)
        hsl = slice(c * HC, (c + 1) * HC)
        # GRU sharded by OUTPUT rows: core c owns gate rows hsl of r/z/n.
        # slab layout per chunk f: cols [r(128) | z(128) | n(128)]
        def g_rows(w):
            return np.concatenate([w[hsl], w[H:2 * H][hsl], w[2 * H:][hsl]])
        m["wih_w"] = _pack([_slabs(g_rows(wih).T, 3 * HC)])
        m["whh_w"] = _pack([_slabs(g_rows(whh).T, 3 * HC)])
        m["h0c_row"] = np.ascontiguousarray(h0[hsl].reshape(1, HC))
        gb = np.zeros((128, 4 * HC), np.float32)
        gb[0, 0:HC] = (bih + bhh)[hsl]
        gb[0, HC:2 * HC] = (bih + bhh)[H:2 * H][hsl]
        gb[0, 2 * HC:3 * HC] = bih[2 * H:][hsl]
        gb[0, 3 * HC:4 * HC] = bhh[2 * H:][hsl]
        m["gbias"] = gb.astype(BF16)

        wt_c = owt[:, c * VC:(c + 1) * VC] * OWS
        if OUTW_DT == "fp8dr":
            # mm tiles of 208 cols: [t32, p, pair, j, n], k = p*8 + 2*pair + j
            # packed per DMA slab of two mm tiles -> [16, 128, 3328]
            m["outw"] = np.ascontiguousarray(
                wt_c.reshape(128, 4, 2, 32, 208).transpose(3, 0, 1, 2, 4)
                .reshape(16, 2, 128, 4 * 2 * 208).transpose(0, 2, 1, 3)
                .reshape(NT, 128, FH * TW)).astype(FP8)
        else:
            m["outw"] = np.ascontiguousarray(
                wt_c.reshape(128, FH, NT, TW).transpose(2, 0, 1, 3).reshape(NT, 128, FH * TW)
            ).astype(FP8 if OUTW_DT == "fp8" else BF16)
        m["outb"] = np.ascontiguousarray(ob[c * VC:(c + 1) * VC].reshape(128, FP))
        in_maps.append(m)
    return in_maps


# ------------------------------------------------------------- device kernel

def build_nc():
    bf = mybir.dt.bfloat16
    f8 = mybir.dt.float8e4
    f32 = mybir.dt.float32
    ACT = mybir.ActivationFunctionType
    OP = mybir.AluOpType

    nc = bacc.Bacc("TRN2", target_bir_lowering=False, debug=False, num_devices=NC)

    i_cat1 = nc.dram_tensor("cat1_bf", [128, F2H], bf, kind="ExternalInput")
    i_emb = nc.dram_tensor("emb_bf", [128, FH], bf, kind="ExternalInput")
    i_h0b = nc.dram_tensor("h0_bf", [128, FH], bf, kind="ExternalInput")
    i_h0r = nc.dram_tensor("h0c_row", [1, HC], f32, kind="ExternalInput")
    i_e0 = nc.dram_tensor("e0_bf", [128, 1], bf, kind="ExternalInput")
    i_attn = nc.dram_tensor("attn_w", [128, 17 * L], bf, kind="ExternalInput")
    i_enc = nc.dram_tensor("enc_w", [128, FL * H], bf, kind="ExternalInput")
    i_comb = nc.dram_tensor("comb_w", [128, 17 * H], bf, kind="ExternalInput")
    i_wih = nc.dram_tensor("wih_w", [128, 3 * H], bf, kind="ExternalInput")
    i_whh = nc.dram_tensor("whh_w", [128, 3 * H], bf, kind="ExternalInput")
    i_gb = nc.dram_tensor("gbias", [128, 4 * HC], bf, kind="ExternalInput")
    wdt = f8 if OUTW_DT in ("fp8", "fp8dr") else bf
    i_outw = nc.dram_tensor("outw", [NT, 128, FH * TW], wdt, kind="ExternalInput")
    i_outb = nc.dram_tensor("outb", [128, FP], f32, kind="ExternalInput")

    o_logp = nc.dram_tensor("logp", [128, FP], f32, kind="ExternalOutput")
    o_hnew = nc.dram_tensor("hnew", [1, HC], f32, kind="ExternalOutput")
    o_attnw = nc.dram_tensor("attnw", [1, L], f32, kind="ExternalOutput")
    o_dbg = nc.dram_tensor("dbg", [1, 16], f32, kind="ExternalOutput")

    with tile.TileContext(nc) as tc:
        with tc.tile_pool(name="sb", bufs=1) as sb, \
             tc.tile_pool(name="ps", bufs=1, space="PSUM") as ps, \
             tc.tile_pool(name="dram", bufs=1, space="DRAM") as dram:

            def prow(shape, tag, name):
                pad = [1, 1024] if shape[0] == 1 else [128, 256]
                return ps.tile(shape, f32, tag=tag, padded_shape=pad, name=name)

            # ---- dummy AllReduce right away (absorbs ncfw startup)
            warm_in = dram.tile([1, 4], f32)
            warm_out = dram.tile([NC, 4], f32)
            nc.gpsimd.collective_compute(
                "AllGather", OP.bypass, replica_groups=[list(range(NC))],
                ins=[warm_in.opt()], outs=[warm_out.opt()])

            # ---- ACT table pre-warm (Exp for attention; others staged later)
            warm1 = sb.tile([1, 1], f32)
            nc.vector.memset(warm1[:], 1.0)
            wtmp = sb.tile([1, 1], f32)
            nc.scalar.activation(wtmp[:], warm1[:], ACT.Exp)

            # ---- inputs -> SBUF (all resident), critical-path order
            cat1_bf = sb.tile([128, F2H], bf)
            nc.sync.dma_start(cat1_bf[:], i_cat1[:])
            emb_bf = sb.tile([128, FH], bf)
            nc.sync.dma_start(emb_bf[:], i_emb[:])
            h0_bf = sb.tile([128, FH], bf)
            nc.sync.dma_start(h0_bf[:], i_h0b[:])
            h0c_row = sb.tile([1, HC], f32)
            nc.sync.dma_start(h0c_row[:], i_h0r[:])
            e0_bf = sb.tile([128, 1], bf)
            nc.sync.dma_start(e0_bf[:], i_e0[:])
            attn_sb = sb.tile([128, 17 * L], bf)
            for q in range(8):
                s = slice(q * 1088, (q + 1) * 1088)
                nc.sync.dma_start(attn_sb[:, s], i_attn[:, s])
            enc_sb = sb.tile([128, FL * H], bf)
            for q in range(4):
                s = slice(q * 1024, (q + 1) * 1024)
                nc.sync.dma_start(enc_sb[:, s], i_enc[:, s])
            comb_sb = sb.tile([128, 17 * H], bf)
            for q in range(8):
                s = slice(q * 2176, (q + 1) * 2176)
                nc.sync.dma_start(comb_sb[:, s], i_comb[:, s])
            wih_sb = sb.tile([128, 3 * H], bf)
            for q in range(2):
                s = slice(q * 1536, (q + 1) * 1536)
                nc.sync.dma_start(wih_sb[:, s], i_wih[:, s])
            whh_sb = sb.tile([128, 3 * H], bf)
            for q in range(2):
                s = slice(q * 1536, (q + 1) * 1536)
                nc.sync.dma_start(whh_sb[:, s], i_whh[:, s])
            gb_sb = sb.tile([128, 4 * HC], bf)
            nc.sync.dma_start(gb_sb[:], i_gb[:])
            outb_pf = sb.tile([128, FP], f32)
            nc.sync.dma_start(outb_pf[:], i_outb[:])

            outw_tiles = []
            for t in range(NT):
                w = sb.tile([128, FH * TW], wdt, tag="ow", bufs=NT, name=f"ow{t}")
                nc.sync.dma_start(w[:], i_outw[t])
                outw_tiles.append(w)

            ones128 = sb.tile([128, 1], f32)
            nc.vector.memset(ones128[:], 1.0)
            ones_row = sb.tile([1, 128], f32)
            nc.vector.memset(ones_row[:], 1.0)

            # ================= attention (replicated) =================
            att_ps = prow([1, L], "g0", "att_ps")
            for f in range(F2H):
                nc.tensor.matmul(att_ps[:], cat1_bf[:, f:f + 1],
                                 attn_sb[:, f * L:(f + 1) * L],
                                 start=(f == 0), stop=False)
            nc.tensor.matmul(att_ps[:], e0_bf[:], attn_sb[:, 16 * L:17 * L],
                             start=False, stop=True)
            ew_row = sb.tile([1, L], f32)
            sA = sb.tile([1, 1], f32)
            nc.scalar.activation(ew_row[:], att_ps[:], ACT.Exp, accum_out=sA[:])
            rA = sb.tile([1, 1], f32)
            nc.vector.reciprocal(rA[:], sA[:])
            aw_row = sb.tile([1, L], f32)
            nc.vector.tensor_scalar_mul(aw_row[:], ew_row[:], rA[:])
            nc.gpsimd.dma_start(o_attnw[:], aw_row[:])

            nc.scalar.activation(wtmp[:], warm1[:], ACT.Sigmoid)
            nc.scalar.activation(wtmp[:], warm1[:], ACT.Tanh)
            ew_pf = sb.tile([128, FL], f32)
            nc.gpsimd.dma_start(ew_pf[:], ew_row[:])
            ew_bf = sb.tile([128, FL], bf)
            nc.vector.tensor_copy(ew_bf[:], ew_pf[:])

            ctx_ps = prow([1, H], "g1", "ctx_ps")
            for nt2 in range(2):
                cs = slice(nt2 * 512, (nt2 + 1) * 512)
                for f in range(FL):
                    nc.tensor.matmul(ctx_ps[0:1, cs], ew_bf[:, f:f + 1],
                                     enc_sb[:, f * H + nt2 * 512:f * H + (nt2 + 1) * 512],
                                     start=(f == 0), stop=(f == FL - 1))
            ctx_row = sb.tile([1, H], f32)
            nc.scalar.mul(ctx_row[:], ctx_ps[:], rA[0:1, 0:1])
            ctx_pf = sb.tile([128, FH], f32)
            nc.gpsimd.dma_start(ctx_pf[:], ctx_row[:])
            ctx_bf = sb.tile([128, FH], bf)
            nc.vector.tensor_copy(ctx_bf[:], ctx_pf[:])

            # ================= combine (replicated) =================
            x_ps = prow([1, H], "g2", "x_ps")
            for nt2 in range(2):
                cs = slice(nt2 * 512, (nt2 + 1) * 512)
                for f in range(FH):
                    nc.tensor.matmul(x_ps[0:1, cs], emb_bf[:, f:f + 1],
                                     comb_sb[:, f * H + nt2 * 512:f * H + (nt2 + 1) * 512],
                                     start=(f == 0), stop=False)
                for f in range(FH):
                    nc.tensor.matmul(x_ps[0:1, cs], ctx_bf[:, f:f + 1],
                                     comb_sb[:, (8 + f) * H + nt2 * 512:(8 + f) * H + (nt2 + 1) * 512],
                                     start=False, stop=False)
                nc.tensor.matmul(x_ps[0:1, cs], e0_bf[:],
                                 comb_sb[:, 16 * H + nt2 * 512:16 * H + (nt2 + 1) * 512],
                                 start=False, stop=True)
            x_row = sb.tile([1, H], f32)
            nc.scalar.activation(x_row[:], x_ps[:], ACT.Relu)
            x_pf = sb.tile([128, FH], f32)
            nc.gpsimd.dma_start(x_pf[:], x_row[:])
            x_bf = sb.tile([128, FH], bf)
            nc.vector.tensor_copy(x_bf[:], x_pf[:])

            # ================= GRU partials (+bias on core0) =================
            # payload [r (1024) | z (1024) | n_i (1024) | n_h (1024)]
            def part_psum(tag, name, wcol, use_x, use_h, bcol):
                gp = prow([1, H], tag, name)
                for nt2 in range(2):
                    cs = slice(nt2 * 512, (nt2 + 1) * 512)
                    ws = slice(wcol + nt2 * 512, wcol + (nt2 + 1) * 512)
                    first = True
                    if use_x:
                        nc.tensor.matmul(gp[0:1, cs], x128_bf[:], wih_sb[:, ws],
                                         start=True, stop=False)
                        first = False
                    if use_h:
                        nc.tensor.matmul(gp[0:1, cs], h0c_bf[:], whh_sb[:, ws],
                                         start=first, stop=False)
                    nc.tensor.matmul(gp[0:1, cs], e0_bf[:],
                                     gb_sb[:, bcol + nt2 * 512:bcol + (nt2 + 1) * 512],
                                     start=False, stop=True)
                return gp

            r_ps = part_psum("g2", "r_ps", 0, True, True, 0)
            z_ps = part_psum("g3", "z_ps", H, True, True, H)
            ni_ps = part_psum("g0", "ni_ps", 2 * H, True, False, 2 * H)
            nh_ps = part_psum("g1", "nh_ps", 2 * H, False, True, 3 * H)

            pay2 = sb.tile([1, 4096], f32)
            nc.vector.tensor_copy(pay2[0:1, 0:1024], r_ps[:])
            nc.scalar.copy(pay2[0:1, 1024:2048], z_ps[:])
            nc.vector.tensor_copy(pay2[0:1, 2048:3072], ni_ps[:])
            nc.scalar.copy(pay2[0:1, 3072:4096], nh_ps[:])

            cc2_in = dram.tile([1, 4096], f32)
            cc2_out = dram.tile([1, 4096], f32)
            nc.gpsimd.dma_start(cc2_in[:], pay2[:])
            nc.gpsimd.collective_compute(
                "AllReduce", OP.add, replica_groups=[list(range(NC))],
                ins=[cc2_in.opt()], outs=[cc2_out.opt()])

            # gates in pf layout straight from the AllReduce result
            r_pf = sb.tile([128, FH], f32)
            nc.gpsimd.dma_start(r_pf[:], cc2_out[0:1, 0:1024])
            z_pf = sb.tile([128, FH], f32)
            nc.gpsimd.dma_start(z_pf[:], cc2_out[0:1, 1024:2048])
            ni_pf = sb.tile([128, FH], f32)
            nc.gpsimd.dma_start(ni_pf[:], cc2_out[0:1, 2048:3072])
            nh_pf = sb.tile([128, FH], f32)
            nc.gpsimd.dma_start(nh_pf[:], cc2_out[0:1, 3072:4096])

            r_s = sb.tile([128, FH], f32)
            nc.scalar.activation(r_s[:], r_pf[:], ACT.Sigmoid)
            z_s = sb.tile([128, FH], f32)
            nc.scalar.activation(z_s[:], z_pf[:], ACT.Sigmoid)
            rnh = sb.tile([128, FH], f32)
            nc.vector.tensor_mul(rnh[:], r_s[:], nh_pf[:])
            pre_n = sb.tile([128, FH], f32)
            nc.vector.tensor_add(pre_n[:], rnh[:], ni_pf[:])
            n_pf = sb.tile([128, FH], f32)
            nc.scalar.activation(n_pf[:], pre_n[:], ACT.Tanh)
            d_pf = sb.tile([128, FH], f32)
            nc.vector.tensor_sub(d_pf[:], h0_pf[:], n_pf[:])
            zd_pf = sb.tile([128, FH], f32)
            nc.vector.tensor_mul(zd_pf[:], z_s[:], d_pf[:])
            hnew_pf = sb.tile([128, FH], f32)
            nc.vector.tensor_add(hnew_pf[:], n_pf[:], zd_pf[:])
            nc.gpsimd.dma_start(o_hnew[:], hnew_pf[:])
            h_bf = sb.tile([128, FH], wdt)
            nc.vector.tensor_copy(h_bf[:], hnew_pf[:])
            if OUTW_DT == "fp8dr":
                h_dr = sb.tile([128, 128], wdt)
                # col pr*32 + 16*j = h[p*8 + 2*pr + j]
                nc.vector.tensor_copy(h_dr[:, 0:128:32], hnew_pf[:, 0:8:2])
                nc.vector.tensor_copy(h_dr[:, 16:128:32], hnew_pf[:, 1:8:2])

            # ================= output projection (fp8 W, x64) =================
            nc.scalar.activation(wtmp[:], warm1[:], ACT.Exp)
            lg_sb = sb.tile([128, FP], f32)
            for t in range(NT):
                lg_ps = prow([1, TW], f"g{t % 4}", f"lg{t}")
                if OUTW_DT == "fp8dr":
                    wv = outw_tiles[t].rearrange("p (u pr j n) -> p u pr j n",
                                                 u=2, pr=4, j=2)
                    for u in range(2):
                        sub = lg_ps[0:1, u * 208:(u + 1) * 208]
                        for pr in range(4):
                            lhsT = h_dr[:, pr * 32:pr * 32 + 17:16]
                            nc.tensor.matmul(sub, lhsT,
                                             wv[:, u, pr, :, :],
                                             start=(pr == 0), stop=(pr == 3),
                                             perf_mode=mybir.MatmulPerfMode.DoubleRow)
                else:
                    for f in range(FH):
                        nc.tensor.matmul(lg_ps[:], h_bf[:, f:f + 1],
                                         outw_tiles[t][:, f * TW:(f + 1) * TW],
                                         start=(f == 0), stop=(f == FH - 1))
                lg_row = sb.tile([1, TW], f32, tag="lgrow", bufs=4, name=f"lgr{t}")
                if t % 2 == 0:
                    nc.vector.tensor_copy(lg_row[:], lg_ps[:])
                else:
                    nc.scalar.copy(lg_row[:], lg_ps[:])
                nc.gpsimd.dma_start(lg_sb[8 * t:8 * (t + 1), :], lg_row[:])

            # lb = lg / OWS + out_b ; exp + row sums
            lb_sb = sb.tile([128, FP], f32)
            nc.vector.scalar_tensor_tensor(lb_sb[:], lg_sb[:], 1.0 / OWS, outb_pf[:],
                                           op0=mybir.AluOpType.mult,
                                           op1=mybir.AluOpType.add)
            ex_sb = sb.tile([128, FP], f32)
            rowsum = sb.tile([128, 1], f32)
            nc.scalar.activation(ex_sb[:], lb_sb[:], ACT.Exp, accum_out=rowsum[:])

            sum_ps = prow([1, 1], "g1", "sum_ps")
            nc.tensor.matmul(sum_ps[:], ones128[:], rowsum[:], start=True, stop=True)
            s_sb = sb.tile([1, 1], f32)
            nc.scalar.copy(s_sb[:], sum_ps[:])

            nc.scalar.activation(wtmp[:], warm1[:], ACT.Ln)
            cc3_in = dram.tile([1, 1], f32)
            cc3_out = dram.tile([1, 1], f32)
            nc.gpsimd.dma_start(cc3_in[:], s_sb[:])
            nc.gpsimd.collective_compute(
                "AllReduce", OP.add, replica_groups=[list(range(NC))],
                ins=[cc3_in.opt()], outs=[cc3_out.opt()])
            S_sb = sb.tile([1, 1], f32)
            nc.gpsimd.dma_start(S_sb[:], cc3_out[:])

            delta = sb.tile([1, 1], f32)
            nc.scalar.activation(delta[:], S_sb[:], ACT.Ln)
            bc_ps = prow([128, 1], "g2", "bc_ps")
            nc.tensor.matmul(bc_ps[:], ones_row[:], delta[:], start=True, stop=True)
            bc_sb = sb.tile([128, 1], f32)
            nc.vector.tensor_copy(bc_sb[:], bc_ps[:])

            logp_sb = sb.tile([128, FP], f32)
            nc.vector.tensor_scalar(logp_sb[:], lb_sb[:], bc_sb[:], None,
                                    op0=mybir.AluOpType.subtract)
            nc.gpsimd.dma_start(o_logp[:], logp_sb[:])

            warm_sb = sb.tile([NC, 4], f32)
            nc.sync.dma_start(warm_sb[:], warm_out[:])
            nc.sync.dma_start(o_dbg[0:1, 0:4], warm_sb[0:1, :])

    nc.compile()
    return nc


# ------------------------------------------------------------------- runner

def _get_nc():
    if "nc" not in _CACHE:
        _CACHE["nc"] = build_nc()
    return _CACHE["nc"]


def kernel(**inputs):
    global LAST_EXEC_NS
    in_maps = prep_inputs(**inputs)
    nc = _get_nc()
    trace = bool(int(os.environ.get("KERNEL_TRACE", "0")))
    if trace:
        try:
            from bass_exec import run_spmd_traced
            res = run_spmd_traced(nc, in_maps, NC)
        except Exception:
            res = bass_utils.run_bass_kernel_spmd(
                nc, in_maps, core_ids=list(range(NC)))
    else:
        res = bass_utils.run_bass_kernel_spmd(
            nc, in_maps, core_ids=list(range(NC)))
    LAST_EXEC_NS = res.exec_time_ns

    logp = np.concatenate(
        [res.results[c]["logp"].reshape(VC) for c in range(NC)])[:V][None, :]
    hnew = res.results[0]["hnew"].reshape(1, 1, H)
    attnw = res.results[0]["attnw"].reshape(1, L)
    return (np.ascontiguousarray(logp.astype(np.float32)),
            np.ascontiguousarray(hnew.astype(np.float32)),
            np.ascontiguousarray(attnw.astype(np.float32)))
